# revision 5
# baseline (speedup 1.0000x reference)
"""AdaptiveLiquidNeuron forward on 8 TRN2 NeuronCores (data-parallel over batch).

Math (per batch row, H=1024):
  context = relu(h @ W1.T + b1) @ W2.T + b2
  pa      = context @ PM.T + pm_b
  mm      = (1 + pa) * (e @ Wrec.T)
  dh      = (-decay*h + mm + bias) / (tau * sigmoid(pa))
  out     = LayerNorm(dh) * ln_w + ln_b

Strategy: shard B=16384 over 8 cores (2048 rows each), replicate H x H weights.
On-chip everything is kept transposed ([H on partitions, B on free]) so the four
matmuls need no on-chip transposes (host pre-transposes weights + activations).
LayerNorm reduces over the partition axis via ones-matmuls; stats are broadcast
back across partitions with K=1 matmuls. Host folds 1/tau into Wrec/decay/bias,
ce_b2 into pm_b, and uses 1/sigmoid(x) = 1 + exp(-x).
"""

import numpy as np
import ml_dtypes

BF16 = ml_dtypes.bfloat16

B, H = 16384, 1024
NCORES = 8
BL = B // NCORES      # 2048 batch rows per core
P = 128               # partitions
KC = H // P           # 8 chunks of the hidden dim
NB = 8                # batch tiles per core
NT = BL // NB         # 256 batch columns per tile
EPS = 1e-5

# consts layout: [128, 6*KC] f32, column v*KC + m = chunk m of vector v
V_B1, V_PMB, V_NDEC, V_BIASP, V_LNW, V_LNB = range(6)

_CACHED = {}


def _build_nc():
    import concourse.bacc as bacc
    import concourse.tile as tile
    from concourse import mybir
    from contextlib import ExitStack

    f32 = mybir.dt.float32
    bf16 = mybir.dt.bfloat16
    AF = mybir.ActivationFunctionType
    OP = mybir.AluOpType

    nc = bacc.Bacc(target_bir_lowering=False)

    hT_e = nc.declare_dram_parameter("hT", [H, BL], bf16, isOutput=False)
    eT_e = nc.declare_dram_parameter("eT", [H, BL], bf16, isOutput=False)
    w1_e = nc.declare_dram_parameter("w1T", [H, H], bf16, isOutput=False)
    w2_e = nc.declare_dram_parameter("w2T", [H, H], bf16, isOutput=False)
    pm_e = nc.declare_dram_parameter("pmT", [H, H], bf16, isOutput=False)
    wr_e = nc.declare_dram_parameter("wrT", [H, H], bf16, isOutput=False)
    cs_e = nc.declare_dram_parameter("consts", [P, 6 * KC], f32, isOutput=False)
    out_e = nc.declare_dram_parameter("out", [H, BL], f32, isOutput=True)

    hT_r = hT_e[:].rearrange("(k p) b -> p k b", p=P)
    eT_r = eT_e[:].rearrange("(k p) b -> p k b", p=P)
    out_r = out_e[:].rearrange("(m p) b -> p m b", p=P)

    with tile.TileContext(nc) as tc, ExitStack() as ctx:
        wpool = ctx.enter_context(tc.tile_pool(name="weights", bufs=1))
        cpool = ctx.enter_context(tc.tile_pool(name="consts", bufs=1))
        iopool = ctx.enter_context(tc.tile_pool(name="io", bufs=2))
        actpool = ctx.enter_context(tc.tile_pool(name="acts", bufs=1))
        fpool = ctx.enter_context(tc.tile_pool(name="f32work", bufs=1))
        dhpool = ctx.enter_context(tc.tile_pool(name="dh", bufs=2))
        rpool = ctx.enter_context(tc.tile_pool(name="redu", bufs=2))
        ypool = ctx.enter_context(tc.tile_pool(name="y", bufs=4))
        rowpool = ctx.enter_context(tc.tile_pool(name="rows", bufs=2))
        outpool = ctx.enter_context(tc.tile_pool(name="outs", bufs=2))
        psA = ctx.enter_context(tc.tile_pool(name="psA", bufs=3, space="PSUM"))
        psS = ctx.enter_context(tc.tile_pool(name="psS", bufs=1, space="PSUM"))
        psB = ctx.enter_context(tc.tile_pool(name="psB", bufs=1, space="PSUM"))

        # ---- resident constants / weights ----
        w_sb = []
        for nm, ext in (("w1", w1_e), ("w2", w2_e), ("pm", pm_e), ("wr", wr_e)):
            t = wpool.tile([P, KC, H], bf16, tag=nm)
            nc.sync.dma_start(out=t[:], in_=ext[:].rearrange("(k p) m -> p k m", p=P))
            w_sb.append(t)
        w1_sb, w2_sb, pm_sb, wr_sb = w_sb

        consts = cpool.tile([P, 6 * KC], f32, tag="consts")
        nc.sync.dma_start(out=consts[:], in_=cs_e[:])

        def col(v, m):
            return consts[:, v * KC + m : v * KC + m + 1]

        ones_col = cpool.tile([P, 1], bf16, tag="ones_col")
        nc.vector.memset(ones_col[:], 1.0)
        ones_row = cpool.tile([1, P], f32, tag="ones_row")
        nc.vector.memset(ones_row[:], 1.0)
        eps_t = cpool.tile([1, 1], f32, tag="eps")
        nc.vector.memset(eps_t[:], EPS)

        # ---- per-b-tile state handed from matmul phase to epilogue ----
        state = [None] * NB

        def load_io(i):
            ht = iopool.tile([P, KC, NT], bf16, tag="hT")
            et = iopool.tile([P, KC, NT], bf16, tag="eT")
            nc.sync.dma_start(out=ht[:], in_=hT_r[:, :, i * NT : (i + 1) * NT])
            nc.sync.dma_start(out=et[:], in_=eT_r[:, :, i * NT : (i + 1) * NT])
            return ht, et

        io_tiles = [load_io(0), load_io(1)]

        def mm_layer(w, rhs_t, evac):
            """out[m] = evac(m, psum) for psum = w[:,:,m].T @ rhs (contract KC chunks)."""
            for m in range(KC):
                acc = psA.tile([P, NT], f32, tag="acc")
                for k in range(KC):
                    nc.tensor.matmul(
                        acc[:],
                        w[:, k, m * P : (m + 1) * P],
                        rhs_t[:, k, :],
                        start=(k == 0),
                        stop=(k == KC - 1),
                    )
                evac(m, acc)

        def matmul_phase(i, pe_hook1, pe_hook2):
            ht, et = io_tiles[i % 2]
            if i + 1 < NB:
                io_tiles[(i + 1) % 2] = load_io(i + 1)

            c1 = actpool.tile([P, KC, NT], bf16, tag="c1")
            cx = actpool.tile([P, KC, NT], bf16, tag="ctx")
            pa = fpool.tile([P, KC, NT], f32, tag="pa")
            ex = fpool.tile([P, KC, NT], f32, tag="exp")
            t2 = fpool.tile([P, KC, NT], f32, tag="t2")
            u = fpool.tile([P, KC, NT], f32, tag="u")
            num = fpool.tile([P, KC, NT], f32, tag="num")
            dh = dhpool.tile([P, KC, NT], f32, tag="dh")
            dh_bf = rpool.tile([P, KC, NT], bf16, tag="dh_bf")
            sq_bf = rpool.tile([P, KC, NT], bf16, tag="sq_bf")

            # context encoder layer 1: c1 = relu(W1 @ hT + b1)
            mm_layer(
                w1_sb,
                ht,
                lambda m, acc: nc.scalar.activation(
                    c1[:, m, :], acc[:], AF.Relu, bias=col(V_B1, m), scale=1.0
                ),
            )
            pe_hook1()  # reductions of tile i-1 slot in here on PE
            # context encoder layer 2 (b2 folded into pm_b): ctx = W2 @ c1
            mm_layer(
                w2_sb,
                c1,
                lambda m, acc: nc.scalar.activation(
                    cx[:, m, :], acc[:], AF.Copy, bias=0.0, scale=1.0
                ),
            )
            pe_hook2()  # stat broadcasts of tile i-1
            # param modulator: pa = PM @ ctx + pm_b'
            mm_layer(
                pm_sb,
                cx,
                lambda m, acc: nc.vector.tensor_scalar_add(
                    pa[:, m, :], acc[:], col(V_PMB, m)
                ),
            )
            # 1/sigmoid(pa) = 1 + exp(-pa)
            nc.scalar.activation(ex[:], pa[:], AF.Exp, bias=0.0, scale=-1.0)

            # recurrent: t2 = (1 + pa) * (Wrec' @ eT); u = -decay'*h + bias'
            def evac4(m, acc):
                nc.vector.scalar_tensor_tensor(
                    t2[:, m, :], pa[:, m, :], 1.0, acc[:], op0=OP.add, op1=OP.mult
                )
                nc.vector.tensor_scalar(
                    u[:, m, :],
                    ht[:, m, :],
                    col(V_NDEC, m),
                    col(V_BIASP, m),
                    op0=OP.mult,
                    op1=OP.add,
                )

            mm_layer(wr_sb, et, evac4)

            nc.vector.tensor_add(num[:], t2[:], u[:])
            # dh = num * (1 + exp(-pa))
            nc.vector.scalar_tensor_tensor(
                dh[:], ex[:], 1.0, num[:], op0=OP.add, op1=OP.mult
            )
            nc.scalar.activation(dh_bf[:], dh[:], AF.Copy, bias=0.0, scale=1.0)
            nc.vector.tensor_tensor(sq_bf[:], dh[:], dh[:], op=OP.mult)
            state[i] = (dh, dh_bf, sq_bf)

        def reduce_phase(i):
            # partition-axis sums via ones-matmul: sum/sumsq over all H=1024
            dh, dh_bf, sq_bf = state[i]
            sum_ps = psS.tile([1, NT], f32, tag="sum")
            sq_ps = psS.tile([1, NT], f32, tag="sumsq")
            for m in range(KC):
                nc.tensor.matmul(
                    sum_ps[:], ones_col[:], dh_bf[:, m, :],
                    start=(m == 0), stop=(m == KC - 1),
                )
            for m in range(KC):
                nc.tensor.matmul(
                    sq_ps[:], ones_col[:], sq_bf[:, m, :],
                    start=(m == 0), stop=(m == KC - 1),
                )
            mu = rowpool.tile([1, NT], f32, tag="mu")
            ms = rowpool.tile([1, NT], f32, tag="ms")
            var = rowpool.tile([1, NT], f32, tag="var")
            std = rowpool.tile([1, NT], f32, tag="std")
            rstd = rowpool.tile([1, NT], f32, tag="rstd")
            qrow = rowpool.tile([1, NT], f32, tag="qrow")
            nc.vector.tensor_scalar_mul(mu[:], sum_ps[:], 1.0 / H)
            nc.vector.tensor_scalar_mul(ms[:], sq_ps[:], 1.0 / H)
            # var = E[x^2] - mu^2
            musq = rowpool.tile([1, NT], f32, tag="musq")
            nc.vector.tensor_mul(musq[:], mu[:], mu[:])
            nc.vector.tensor_sub(var[:], ms[:], musq[:])
            nc.scalar.activation(std[:], var[:], AF.Sqrt, bias=eps_t[:], scale=1.0)
            nc.vector.reciprocal(rstd[:], std[:])
            nc.vector.tensor_mul(qrow[:], mu[:], rstd[:])
            state[i] = (dh, rstd, qrow)

        def bcast_phase(i):
            dh, rstd, qrow = state[i]
            p_ps = psB.tile([P, NT], f32, tag="P")
            q_ps = psB.tile([P, NT], f32, tag="Q")
            nc.tensor.matmul(p_ps[:], ones_row[:], rstd[:], start=True, stop=True)
            nc.tensor.matmul(q_ps[:], ones_row[:], qrow[:], start=True, stop=True)
            state[i] = (dh, p_ps, q_ps)

        def epilogue_phase(i):
            dh, p_ps, q_ps = state[i]
            outf = outpool.tile([P, KC, NT], f32, tag="outf")
            for m in range(KC):
                y1 = ypool.tile([P, NT], f32, tag="y1")
                y2 = ypool.tile([P, NT], f32, tag="y2")
                nc.vector.tensor_mul(y1[:], dh[:, m, :], p_ps[:])
                nc.vector.tensor_sub(y2[:], y1[:], q_ps[:])
                nc.vector.tensor_scalar(
                    outf[:, m, :], y2[:], col(V_LNW, m), col(V_LNB, m),
                    op0=OP.mult, op1=OP.add,
                )
            nc.sync.dma_start(out=out_r[:, :, i * NT : (i + 1) * NT], in_=outf[:])
            state[i] = None

        for i in range(NB):
            matmul_phase(
                i,
                (lambda j=i: reduce_phase(j - 1)) if i > 0 else (lambda: None),
                (lambda j=i: bcast_phase(j - 1)) if i > 0 else (lambda: None),
            )
            if i > 0:
                epilogue_phase(i - 1)
        reduce_phase(NB - 1)
        bcast_phase(NB - 1)
        epilogue_phase(NB - 1)

    if not nc.is_finalized():
        nc.finalize()
    return nc


def _get_nc():
    if "nc" not in _CACHED:
        _CACHED["nc"] = _build_nc()
    return _CACHED["nc"]


# test.py can flip these before calling kernel() to profile
TRACE = False
LAST_RESULT = {}


def kernel(t, h, e, W_rec, bias, tau, decay, ln_w, ln_b,
           ce_w1, ce_b1, ce_w2, ce_b2, pm_w, pm_b):
    from concourse.bass_utils import run_bass_kernel_spmd

    f = np.float32
    h = np.asarray(h, f)
    e = np.asarray(e, f)
    W_rec = np.asarray(W_rec, f)
    bias = np.asarray(bias, f)
    tau = np.asarray(tau, f)
    decay = np.asarray(decay, f)
    ln_w = np.asarray(ln_w, f)
    ln_b = np.asarray(ln_b, f)
    ce_w1 = np.asarray(ce_w1, f)
    ce_b1 = np.asarray(ce_b1, f)
    ce_w2 = np.asarray(ce_w2, f)
    ce_b2 = np.asarray(ce_b2, f)
    pm_w = np.asarray(pm_w, f)
    pm_b = np.asarray(pm_b, f)

    invtau = 1.0 / tau
    negdecay = -decay * invtau
    biasp = bias * invtau
    pmb_eff = pm_b + pm_w @ ce_b2  # fold ce_b2 through the param modulator

    w1T = np.ascontiguousarray(ce_w1.T).astype(BF16)
    w2T = np.ascontiguousarray(ce_w2.T).astype(BF16)
    pmT = np.ascontiguousarray(pm_w.T).astype(BF16)
    wrT = np.ascontiguousarray(W_rec.T * invtau[None, :]).astype(BF16)

    def chunked(v):  # [H] -> [128, KC] with column m = chunk m
        return np.ascontiguousarray(v.reshape(KC, P).T)

    consts = np.concatenate(
        [chunked(v) for v in (ce_b1, pmb_eff, negdecay, biasp, ln_w, ln_b)], axis=1
    ).astype(f)

    in_maps = []
    for i in range(NCORES):
        rows = slice(i * BL, (i + 1) * BL)
        in_maps.append({
            "hT": np.ascontiguousarray(h[rows].T).astype(BF16),
            "eT": np.ascontiguousarray(e[rows].T).astype(BF16),
            "w1T": w1T, "w2T": w2T, "pmT": pmT, "wrT": wrT,
            "consts": consts,
        })

    nc = _get_nc()
    res = run_bass_kernel_spmd(nc, in_maps, core_ids=list(range(NCORES)),
                               trace=TRACE)
    LAST_RESULT["exec_time_ns"] = res.exec_time_ns
    LAST_RESULT["mean_exec_time_ns"] = res.mean_exec_time_ns

    out = np.empty((B, H), f)
    for i in range(NCORES):
        out[i * BL : (i + 1) * BL] = res.results[i]["out"].T
    return out


# revision 6
# speedup vs baseline: 1.0013x; 1.0013x over previous
"""AdaptiveLiquidNeuron forward on 8 TRN2 NeuronCores (data-parallel over batch).

Math (per batch row, H=1024):
  context = relu(h @ W1.T + b1) @ W2.T + b2
  pa      = context @ PM.T + pm_b
  mm      = (1 + pa) * (e @ Wrec.T)
  dh      = (-decay*h + mm + bias) / (tau * sigmoid(pa))
  out     = LayerNorm(dh) * ln_w + ln_b

Strategy: shard B=16384 over 8 cores (2048 rows each), replicate H x H weights.
On-chip everything is kept transposed ([H on partitions, B on free]) so the four
matmuls need no on-chip transposes (host pre-transposes weights + activations).
LayerNorm reduces over the partition axis via ones-matmuls; stats are broadcast
back across partitions with K=1 matmuls. Host folds 1/tau into Wrec/decay/bias,
ce_b2 into pm_b, and uses 1/sigmoid(x) = 1 + exp(-x).
"""

import numpy as np
import ml_dtypes

BF16 = ml_dtypes.bfloat16

B, H = 16384, 1024
NCORES = 8
BL = B // NCORES      # 2048 batch rows per core
P = 128               # partitions
KC = H // P           # 8 chunks of the hidden dim
NB = 8                # batch tiles per core
NT = BL // NB         # 256 batch columns per tile
EPS = 1e-5

# consts layout: [128, 6*KC] f32, column v*KC + m = chunk m of vector v
V_B1, V_PMB, V_NDEC, V_BIASP, V_LNW, V_LNB = range(6)

_CACHED = {}


def _build_nc():
    import concourse.bacc as bacc
    import concourse.tile as tile
    from concourse import mybir
    from contextlib import ExitStack

    f32 = mybir.dt.float32
    bf16 = mybir.dt.bfloat16
    AF = mybir.ActivationFunctionType
    OP = mybir.AluOpType

    nc = bacc.Bacc(target_bir_lowering=False)

    hT_e = nc.declare_dram_parameter("hT", [H, BL], bf16, isOutput=False)
    eT_e = nc.declare_dram_parameter("eT", [H, BL], bf16, isOutput=False)
    w1_e = nc.declare_dram_parameter("w1T", [H, H], bf16, isOutput=False)
    w2_e = nc.declare_dram_parameter("w2T", [H, H], bf16, isOutput=False)
    pm_e = nc.declare_dram_parameter("pmT", [H, H], bf16, isOutput=False)
    wr_e = nc.declare_dram_parameter("wrT", [H, H], bf16, isOutput=False)
    cs_e = nc.declare_dram_parameter("consts", [P, 6 * KC], f32, isOutput=False)
    out_e = nc.declare_dram_parameter("out", [H, BL], f32, isOutput=True)

    hT_r = hT_e[:].rearrange("(k p) b -> p k b", p=P)
    eT_r = eT_e[:].rearrange("(k p) b -> p k b", p=P)
    out_r = out_e[:].rearrange("(m p) b -> p m b", p=P)

    with tile.TileContext(nc) as tc, ExitStack() as ctx:
        wpool = ctx.enter_context(tc.tile_pool(name="weights", bufs=1))
        cpool = ctx.enter_context(tc.tile_pool(name="consts", bufs=1))
        iopool = ctx.enter_context(tc.tile_pool(name="io", bufs=2))
        actpool = ctx.enter_context(tc.tile_pool(name="acts", bufs=1))
        fpool = ctx.enter_context(tc.tile_pool(name="f32work", bufs=1))
        dhpool = ctx.enter_context(tc.tile_pool(name="dh", bufs=2))
        rpool = ctx.enter_context(tc.tile_pool(name="redu", bufs=2))
        ypool = ctx.enter_context(tc.tile_pool(name="y", bufs=4))
        rowpool = ctx.enter_context(tc.tile_pool(name="rows", bufs=2))
        outpool = ctx.enter_context(tc.tile_pool(name="outs", bufs=2))
        psA = ctx.enter_context(tc.tile_pool(name="psA", bufs=3, space="PSUM"))
        psS = ctx.enter_context(tc.tile_pool(name="psS", bufs=1, space="PSUM"))
        psB = ctx.enter_context(tc.tile_pool(name="psB", bufs=1, space="PSUM"))

        # ---- resident constants / weights ----
        w_sb = []
        for nm, ext in (("w1", w1_e), ("w2", w2_e), ("pm", pm_e), ("wr", wr_e)):
            t = wpool.tile([P, KC, H], bf16, tag=nm)
            nc.sync.dma_start(out=t[:], in_=ext[:].rearrange("(k p) m -> p k m", p=P))
            w_sb.append(t)
        w1_sb, w2_sb, pm_sb, wr_sb = w_sb

        consts = cpool.tile([P, 6 * KC], f32, tag="consts")
        nc.sync.dma_start(out=consts[:], in_=cs_e[:])

        def col(v, m):
            return consts[:, v * KC + m : v * KC + m + 1]

        ones_col = cpool.tile([P, 1], bf16, tag="ones_col")
        nc.vector.memset(ones_col[:], 1.0)
        ones_row = cpool.tile([1, P], f32, tag="ones_row")
        nc.vector.memset(ones_row[:], 1.0)
        eps_t = cpool.tile([1, 1], f32, tag="eps")
        nc.vector.memset(eps_t[:], EPS)

        # ---- per-b-tile state handed from matmul phase to epilogue ----
        state = [None] * NB

        def load_io(i):
            ht = iopool.tile([P, KC, NT], bf16, tag="hT")
            et = iopool.tile([P, KC, NT], bf16, tag="eT")
            nc.sync.dma_start(out=ht[:], in_=hT_r[:, :, i * NT : (i + 1) * NT])
            nc.sync.dma_start(out=et[:], in_=eT_r[:, :, i * NT : (i + 1) * NT])
            return ht, et

        io_tiles = [load_io(0), load_io(1)]

        def mm_layer(w, rhs_t, evac):
            """out[m] = evac(m, psum) for psum = w[:,:,m].T @ rhs (contract KC chunks)."""
            for m in range(KC):
                acc = psA.tile([P, NT], f32, tag="acc")
                for k in range(KC):
                    nc.tensor.matmul(
                        acc[:],
                        w[:, k, m * P : (m + 1) * P],
                        rhs_t[:, k, :],
                        start=(k == 0),
                        stop=(k == KC - 1),
                    )
                evac(m, acc)

        def matmul_phase(i, pe_hook1, pe_hook2):
            ht, et = io_tiles[i % 2]
            if i + 1 < NB:
                io_tiles[(i + 1) % 2] = load_io(i + 1)

            c1 = actpool.tile([P, KC, NT], bf16, tag="c1")
            cx = actpool.tile([P, KC, NT], bf16, tag="ctx")
            pa = fpool.tile([P, KC, NT], f32, tag="pa")
            ex = fpool.tile([P, KC, NT], f32, tag="exp")
            t2 = fpool.tile([P, KC, NT], f32, tag="t2")
            u = fpool.tile([P, KC, NT], f32, tag="u")
            num = fpool.tile([P, KC, NT], f32, tag="num")
            dh = dhpool.tile([P, KC, NT], f32, tag="dh")
            dh_bf = rpool.tile([P, KC, NT], bf16, tag="dh_bf")
            sq_bf = rpool.tile([P, KC, NT], bf16, tag="sq_bf")

            # context encoder layer 1: c1 = relu(W1 @ hT + b1)
            mm_layer(
                w1_sb,
                ht,
                lambda m, acc: nc.scalar.activation(
                    c1[:, m, :], acc[:], AF.Relu, bias=col(V_B1, m), scale=1.0
                ),
            )
            pe_hook1()  # reductions of tile i-1 slot in here on PE
            # context encoder layer 2 (b2 folded into pm_b): ctx = W2 @ c1
            mm_layer(
                w2_sb,
                c1,
                lambda m, acc: nc.scalar.activation(
                    cx[:, m, :], acc[:], AF.Copy, bias=0.0, scale=1.0
                ),
            )
            pe_hook2()  # stat broadcasts of tile i-1
            # param modulator: pa = PM @ ctx + pm_b'
            mm_layer(
                pm_sb,
                cx,
                lambda m, acc: nc.vector.tensor_scalar_add(
                    pa[:, m, :], acc[:], col(V_PMB, m)
                ),
            )
            # 1/sigmoid(pa) = 1 + exp(-pa)
            nc.scalar.activation(ex[:], pa[:], AF.Exp, bias=0.0, scale=-1.0)

            # recurrent: t2 = (1 + pa) * (Wrec' @ eT); u = -decay'*h + bias'
            def evac4(m, acc):
                nc.vector.scalar_tensor_tensor(
                    t2[:, m, :], pa[:, m, :], 1.0, acc[:], op0=OP.add, op1=OP.mult
                )
                nc.vector.tensor_scalar(
                    u[:, m, :],
                    ht[:, m, :],
                    col(V_NDEC, m),
                    col(V_BIASP, m),
                    op0=OP.mult,
                    op1=OP.add,
                )

            mm_layer(wr_sb, et, evac4)

            nc.vector.tensor_add(num[:], t2[:], u[:])
            # dh = num * (1 + exp(-pa))
            nc.vector.scalar_tensor_tensor(
                dh[:], ex[:], 1.0, num[:], op0=OP.add, op1=OP.mult
            )
            nc.scalar.activation(dh_bf[:], dh[:], AF.Copy, bias=0.0, scale=1.0)
            nc.vector.tensor_tensor(sq_bf[:], dh[:], dh[:], op=OP.mult)
            state[i] = (dh, dh_bf, sq_bf)

        def reduce_phase(i):
            # partition-axis sums via ones-matmul: sum/sumsq over all H=1024
            dh, dh_bf, sq_bf = state[i]
            sum_ps = psS.tile([1, NT], f32, tag="sum")
            sq_ps = psS.tile([1, NT], f32, tag="sumsq")
            for m in range(KC):
                nc.tensor.matmul(
                    sum_ps[:], ones_col[:], dh_bf[:, m, :],
                    start=(m == 0), stop=(m == KC - 1),
                )
            for m in range(KC):
                nc.tensor.matmul(
                    sq_ps[:], ones_col[:], sq_bf[:, m, :],
                    start=(m == 0), stop=(m == KC - 1),
                )
            mu = rowpool.tile([1, NT], f32, tag="mu")
            ms = rowpool.tile([1, NT], f32, tag="ms")
            var = rowpool.tile([1, NT], f32, tag="var")
            std = rowpool.tile([1, NT], f32, tag="std")
            rstd = rowpool.tile([1, NT], f32, tag="rstd")
            qrow = rowpool.tile([1, NT], f32, tag="qrow")
            nc.vector.tensor_scalar_mul(mu[:], sum_ps[:], 1.0 / H)
            nc.vector.tensor_scalar_mul(ms[:], sq_ps[:], 1.0 / H)
            # var = E[x^2] - mu^2
            musq = rowpool.tile([1, NT], f32, tag="musq")
            nc.vector.tensor_mul(musq[:], mu[:], mu[:])
            nc.vector.tensor_sub(var[:], ms[:], musq[:])
            nc.scalar.activation(std[:], var[:], AF.Sqrt, bias=eps_t[:], scale=1.0)
            nc.vector.reciprocal(rstd[:], std[:])
            nc.vector.tensor_mul(qrow[:], mu[:], rstd[:])
            state[i] = (dh, rstd, qrow)

        def bcast_phase(i):
            dh, rstd, qrow = state[i]
            p_ps = psB.tile([P, NT], f32, tag="P")
            q_ps = psB.tile([P, NT], f32, tag="Q")
            nc.tensor.matmul(p_ps[:], ones_row[:], rstd[:], start=True, stop=True)
            nc.tensor.matmul(q_ps[:], ones_row[:], qrow[:], start=True, stop=True)
            state[i] = (dh, p_ps, q_ps)

        def epilogue_phase(i):
            dh, p_ps, q_ps = state[i]
            outf = outpool.tile([P, KC, NT], f32, tag="outf")
            for m in range(KC):
                y1 = ypool.tile([P, NT], f32, tag="y1")
                y2 = ypool.tile([P, NT], f32, tag="y2")
                nc.vector.tensor_mul(y1[:], dh[:, m, :], p_ps[:])
                nc.vector.tensor_sub(y2[:], y1[:], q_ps[:])
                nc.vector.tensor_scalar(
                    outf[:, m, :], y2[:], col(V_LNW, m), col(V_LNB, m),
                    op0=OP.mult, op1=OP.add,
                )
            nc.sync.dma_start(out=out_r[:, :, i * NT : (i + 1) * NT], in_=outf[:])
            state[i] = None

        for i in range(NB):
            matmul_phase(
                i,
                (lambda j=i: reduce_phase(j - 1)) if i > 0 else (lambda: None),
                (lambda j=i: bcast_phase(j - 1)) if i > 0 else (lambda: None),
            )
            if i > 0:
                epilogue_phase(i - 1)
        reduce_phase(NB - 1)
        bcast_phase(NB - 1)
        epilogue_phase(NB - 1)

    if not nc.is_finalized():
        nc.finalize()
    return nc


def _get_nc():
    if "nc" not in _CACHED:
        _CACHED["nc"] = _build_nc()
    return _CACHED["nc"]


# test.py can flip these before calling kernel() to profile
TRACE = False
LAST_RESULT = {}


def kernel(t, h, e, W_rec, bias, tau, decay, ln_w, ln_b,
           ce_w1, ce_b1, ce_w2, ce_b2, pm_w, pm_b):
    from concourse.bass_utils import run_bass_kernel_spmd

    f = np.float32
    h = np.asarray(h, f)
    e = np.asarray(e, f)
    W_rec = np.asarray(W_rec, f)
    bias = np.asarray(bias, f)
    tau = np.asarray(tau, f)
    decay = np.asarray(decay, f)
    ln_w = np.asarray(ln_w, f)
    ln_b = np.asarray(ln_b, f)
    ce_w1 = np.asarray(ce_w1, f)
    ce_b1 = np.asarray(ce_b1, f)
    ce_w2 = np.asarray(ce_w2, f)
    ce_b2 = np.asarray(ce_b2, f)
    pm_w = np.asarray(pm_w, f)
    pm_b = np.asarray(pm_b, f)

    invtau = 1.0 / tau
    negdecay = -decay * invtau
    biasp = bias * invtau
    pmb_eff = pm_b + pm_w @ ce_b2  # fold ce_b2 through the param modulator

    w1T = np.ascontiguousarray(ce_w1.T).astype(BF16)
    w2T = np.ascontiguousarray(ce_w2.T).astype(BF16)
    pmT = np.ascontiguousarray(pm_w.T).astype(BF16)
    wrT = np.ascontiguousarray(W_rec.T * invtau[None, :]).astype(BF16)

    def chunked(v):  # [H] -> [128, KC] with column m = chunk m
        return np.ascontiguousarray(v.reshape(KC, P).T)

    consts = np.concatenate(
        [chunked(v) for v in (ce_b1, pmb_eff, negdecay, biasp, ln_w, ln_b)], axis=1
    ).astype(f)

    in_maps = []
    for i in range(NCORES):
        rows = slice(i * BL, (i + 1) * BL)
        in_maps.append({
            "hT": np.ascontiguousarray(h[rows].T).astype(BF16),
            "eT": np.ascontiguousarray(e[rows].T).astype(BF16),
            "w1T": w1T, "w2T": w2T, "pmT": pmT, "wrT": wrT,
            "consts": consts,
        })

    nc = _get_nc()
    res = run_bass_kernel_spmd(nc, in_maps, core_ids=list(range(NCORES)),
                               trace=TRACE)
    LAST_RESULT["exec_time_ns"] = res.exec_time_ns
    LAST_RESULT["mean_exec_time_ns"] = res.mean_exec_time_ns
    LAST_RESULT["instructions_and_trace"] = res.instructions_and_trace

    out = np.empty((B, H), f)
    for i in range(NCORES):
        out[i * BL : (i + 1) * BL] = res.results[i]["out"].T
    return out
